# revision 10
# baseline (speedup 1.0000x reference)
"""Causal self-attention Trainium2 kernel (B=1, S=4096, E=1024, H=16, D=64).

Sharding: tensor-parallel over heads — 2 heads per core (8 cores).
Each core computes Q/K/V for its 2 heads, causal attention, and a partial
o_proj over its 128 output-feature slice; the host sums the 8 partials.

Device-side layout choices:
  * x is pre-transposed on the host to xT [E, S] (bf16) so every matmul
    contracts over the partition axis with contiguous DMA loads.
  * Q/K are kept transposed in SBUF: qt/kt [128(d of 2 heads), S].
  * Logits are computed transposed: lg[kv, q] = K @ Q.T, one matmul per
    head with the two heads packed into PE row-groups (rows 0-63 / 64-127).
  * exp via ScalarE (scale folded in); no max-subtraction needed since
    logits ~ N(0,1) (exp < ~1e3, fp32/bf16 safe).
  * Causal masking: multiply exp tiles by 0/1 masks on the 4 diagonal
    blocks of each q-tile; off-diagonal blocks are fully allowed/skipped.
  * Softmax denominator: V gets an appended ones-column, so the PV matmul
    (out.T[d, q] accumulation) also accumulates den[q] in PSUM row 64.
  * Normalize: DVE reciprocal + GPSIMD partition_broadcast + DVE multiply.
  * o_proj: attn_outT [128(e), S] is exactly the lhsT needed; partial
    result DMA'd straight from PSUM to DRAM (fp32).
"""

import math
import sys

import numpy as np

for _p in ("/opt/trn_rl_repo", "/opt/trn_rl_repo/concourse"):
    if _p not in sys.path:
        sys.path.insert(0, _p)

import ml_dtypes

BF16 = ml_dtypes.bfloat16

S = 4096
E = 1024
H = 16
D = 64
NCORES = 8
DH = 128  # head dims per core (2 heads x 64)
QT = 512  # query tile (free dim of logits matmuls)
NQ = S // QT  # 8
KB = 128  # kv block (partition dim of logits tiles)
SCALE = 1.0 / math.sqrt(D)

_CACHE = {}


def _build_nc():
    import concourse.tile as tile
    from concourse import bacc, mybir

    dt = mybir.dt
    f32 = dt.float32
    bf16 = dt.bfloat16
    Exp = mybir.ActivationFunctionType.Exp

    nc = bacc.Bacc("TRN2", target_bir_lowering=False, debug=False, num_devices=NCORES)

    xT_d = nc.dram_tensor("xT", [E, S], bf16, kind="ExternalInput")
    wq_d = nc.dram_tensor("wq", [128, 1024], bf16, kind="ExternalInput")
    wk_d = nc.dram_tensor("wk", [128, 1024], bf16, kind="ExternalInput")
    wv_d = nc.dram_tensor("wv", [128, 1024], bf16, kind="ExternalInput")
    wo_d = nc.dram_tensor("wo", [128, 1024], bf16, kind="ExternalInput")
    mask_d = nc.dram_tensor("masks", [KB, 4 * QT], bf16, kind="ExternalInput")
    out_d = nc.dram_tensor("out", [S, E], bf16, kind="ExternalOutput")

    with tile.TileContext(nc) as tc:
        from contextlib import ExitStack

        with ExitStack() as ctx:
            sb = ctx.enter_context(tc.tile_pool(name="sb", bufs=1))
            work = ctx.enter_context(tc.tile_pool(name="work", bufs=3, space="PSUM"))
            pvp = ctx.enter_context(tc.tile_pool(name="pvp", bufs=2, space="PSUM"))
            expp = ctx.enter_context(tc.tile_pool(name="expp", bufs=4))
            normp = ctx.enter_context(tc.tile_pool(name="normp", bufs=2))

            # ---- persistent SBUF tensors + input DMA ----
            xts = []
            for ec in range(8):
                t = sb.tile([128, S], bf16, name=f"xt{ec}", tag=f"xt{ec}")
                nc.sync.dma_start(t[:], xT_d[ec * 128 : (ec + 1) * 128, :])
                xts.append(t)

            wq_sb = sb.tile([128, 1024], bf16, name="wq_sb", tag="wq_sb")
            wk_sb = sb.tile([128, 1024], bf16, name="wk_sb", tag="wk_sb")
            wv_sb = sb.tile([128, 1024], bf16, name="wv_sb", tag="wv_sb")
            wo_sb = sb.tile([128, 1024], bf16, name="wo_sb", tag="wo_sb")
            mask_sb = sb.tile([KB, 4 * QT], bf16, name="mask_sb", tag="mask_sb")
            nc.sync.dma_start(wq_sb[:], wq_d[:])
            nc.sync.dma_start(wk_sb[:], wk_d[:])
            nc.sync.dma_start(wv_sb[:], wv_d[:])
            nc.sync.dma_start(wo_sb[:], wo_d[:])
            nc.sync.dma_start(mask_sb[:], mask_d[:])

            kt = sb.tile([128, S], bf16, name="kt", tag="kt")
            qt = sb.tile([128, S], bf16, name="qt", tag="qt")
            aot = sb.tile([128, S], bf16, name="aot", tag="aot")  # attn_out.T
            # seed tile for the denominator-reciprocal broadcast (rows 0 and
            # 32 are written each q-tile; the rest stay zero from this memset)
            bcseed = sb.tile([64, QT], f32, name="bcseed", tag="bcseed")
            nc.vector.memset(bcseed[:], 0.0)

            # ---- K.T / Q.T projections: [128(d), S] = W @ x.T ----
            for st in range(NQ):
                cols = slice(st * QT, (st + 1) * QT)
                for dst, w in ((kt, wk_sb), (qt, wq_sb)):
                    ps = work.tile([128, QT], f32, name="ps_kq", tag="work")
                    for ec in range(8):
                        nc.tensor.matmul(
                            ps[:],
                            lhsT=w[:, ec * 128 : (ec + 1) * 128],
                            rhs=xts[ec][:, cols],
                            start=(ec == 0),
                            stop=(ec == 7),
                        )
                    nc.vector.tensor_copy(dst[:, cols], ps[:])

            # ---- V projection (normal layout, 32 s-blocks) + ones cols ----
            vts = []
            for i in range(32):
                v = sb.tile([128, 130], bf16, name=f"v{i}", tag=f"v{i}")
                nc.vector.memset(v[:, 64:65], 1.0)
                nc.vector.memset(v[:, 129:130], 1.0)
                ps = work.tile([128, 128], f32, name="ps_v", tag="work")
                for ec in range(8):
                    nc.tensor.matmul(
                        ps[:],
                        lhsT=xts[ec][:, i * 128 : (i + 1) * 128],
                        rhs=wv_sb[:, ec * 128 : (ec + 1) * 128],
                        start=(ec == 0),
                        stop=(ec == 7),
                    )
                nc.vector.tensor_copy(v[:, 0:64], ps[:, 0:64])
                nc.vector.tensor_copy(v[:, 65:129], ps[:, 64:128])
                vts.append(v)

            # ---- attention + o_proj per q-tile ----
            for qi in range(NQ):
                qcols = slice(qi * QT, (qi + 1) * QT)
                n_kb = 4 * (qi + 1)
                pvA = pvp.tile([65, QT], f32, name="pvA", tag="pv")
                pvB = pvp.tile([65, QT], f32, name="pvB", tag="pv")
                for kp in range(n_kb // 2):
                    lgA = work.tile([128, 2 * QT], f32, name="lgA", tag="work")
                    lgB = work.tile([128, 2 * QT], f32, name="lgB", tag="work")
                    for j in (0, 1):
                        kb = 2 * kp + j
                        kvs = slice(kb * KB, (kb + 1) * KB)
                        js = slice(j * QT, (j + 1) * QT)
                        nc.tensor.matmul(
                            lgA[:, js], lhsT=kt[0:64, kvs], rhs=qt[0:64, qcols],
                            start=True, stop=True,
                        )
                        nc.tensor.matmul(
                            lgB[:, js], lhsT=kt[64:128, kvs], rhs=qt[64:128, qcols],
                            start=True, stop=True,
                        )
                    expA = expp.tile([128, 2 * QT], bf16, name="expA", tag="exp")
                    expB = expp.tile([128, 2 * QT], bf16, name="expB", tag="exp")
                    nc.scalar.activation(expA[:], lgA[:], Exp, scale=SCALE)
                    nc.scalar.activation(expB[:], lgB[:], Exp, scale=SCALE)
                    for j in (0, 1):
                        kb = 2 * kp + j
                        js = slice(j * QT, (j + 1) * QT)
                        off = kb - 4 * qi
                        if off >= 0:  # diagonal-band block -> apply 0/1 mask
                            ms = mask_sb[:, off * QT : (off + 1) * QT]
                            nc.vector.tensor_mul(expA[:, js], expA[:, js], ms)
                            nc.vector.tensor_mul(expB[:, js], expB[:, js], ms)
                        nc.tensor.matmul(
                            pvA[:], lhsT=vts[kb][:, 0:65], rhs=expA[:, js],
                            start=(kb == 0), stop=(kb == n_kb - 1),
                            skip_group_check=True,
                        )
                        nc.tensor.matmul(
                            pvB[:], lhsT=vts[kb][:, 65:130], rhs=expB[:, js],
                            start=(kb == 0), stop=(kb == n_kb - 1),
                            skip_group_check=True,
                        )
                # normalize: attn_outT = pv[0:64] / pv[64]
                for pv, r0 in ((pvA, 0), (pvB, 64)):
                    nc.vector.reciprocal(bcseed[0:1, :], pv[64:65, :])
                    nc.vector.tensor_copy(bcseed[32:33, :], bcseed[0:1, :])
                    bcast = normp.tile([64, QT], f32, name="bcast", tag="bcast")
                    nc.vector.stream_shuffle(bcast[:], bcseed[:], [0] * 32)
                    nc.vector.tensor_mul(aot[r0 : r0 + 64, qcols], pv[0:64, :], bcast[:])
                # o_proj for this q-tile's rows; partial out -> DRAM
                for sbi in range(4):
                    srow = qi * QT + sbi * 128
                    for half in range(2):
                        po = work.tile([128, 512], f32, name="po", tag="work")
                        nc.tensor.matmul(
                            po[:],
                            lhsT=aot[:, srow : srow + 128],
                            rhs=wo_sb[:, half * 512 : (half + 1) * 512],
                            start=True, stop=True,
                        )
                        ost = expp.tile([128, 512], bf16, name="ost", tag="ost")
                        nc.vector.tensor_copy(ost[:], po[:])
                        nc.sync.dma_start(
                            out_d[srow : srow + 128, half * 512 : (half + 1) * 512],
                            ost[:],
                        )
    nc.compile()
    return nc


def _host_inputs(x, Wq, Wk, Wv, Wo):
    x2 = np.asarray(x, dtype=np.float32).reshape(S, E)
    xT = np.ascontiguousarray(x2.T).astype(BF16)

    # 0/1 masks for the 4 diagonal-band offsets: allowed iff kv + off*128 <= q
    kv = np.arange(KB)[:, None]
    q = np.arange(QT)[None, :]
    masks = np.concatenate(
        [(kv + off * KB <= q).astype(BF16) for off in range(4)], axis=1
    )
    masks = np.ascontiguousarray(masks)

    in_maps = []
    for c in range(NCORES):
        r = slice(128 * c, 128 * (c + 1))

        def pack(wT):  # [1024(e), 128(d)] -> [128(p), ec*128+d]
            return np.ascontiguousarray(
                wT.reshape(8, 128, 128).transpose(1, 0, 2).reshape(128, 1024)
            ).astype(BF16)

        wq_c = pack(np.asarray(Wq, np.float32)[r, :].T)
        wk_c = pack(np.asarray(Wk, np.float32)[r, :].T)
        wv_c = pack(np.asarray(Wv, np.float32)[r, :].T)
        wo_c = np.ascontiguousarray(np.asarray(Wo, np.float32)[:, r].T).astype(BF16)
        in_maps.append(
            {
                "xT": xT,
                "wq": wq_c,
                "wk": wk_c,
                "wv": wv_c,
                "wo": wo_c,
                "masks": masks,
            }
        )
    return in_maps


def _get_nc():
    if "nc" not in _CACHE:
        _CACHE["nc"] = _build_nc()
    return _CACHE["nc"]


def run(x, Wq, Wk, Wv, Wo, trace=False, trace_kwargs=None):
    """Build+run the SPMD kernel; returns (full_output [S,E] f32, BassKernelResults)."""
    from concourse.bass_utils import run_bass_kernel_spmd

    nc = _get_nc()
    in_maps = _host_inputs(x, Wq, Wk, Wv, Wo)
    res = run_bass_kernel_spmd(
        nc,
        in_maps,
        list(range(NCORES)),
        trace=trace,
        **(trace_kwargs or {}),
    )
    out = np.zeros((S, E), dtype=np.float32)
    for c in range(NCORES):
        out += res.results[c]["out"].astype(np.float32)
    return out, res


def kernel(x, Wq, Wk, Wv, Wo):
    out, _ = run(x, Wq, Wk, Wv, Wo)
    return out.reshape(1, S, E).astype(np.float32)


# revision 12
# speedup vs baseline: 1.6096x; 1.6096x over previous
"""Causal self-attention Trainium2 kernel (B=1, S=4096, E=1024, H=16, D=64).

Sharding: tensor-parallel over heads — 2 heads per core (8 cores).
Each core computes Q/K/V for its 2 heads, causal attention, and a partial
o_proj over its 128 output-feature slice; the host sums the 8 partials.

Device-side structure (per core):
  * x arrives pre-transposed as xT [E, S] bf16 (host does the transpose),
    so every matmul contracts over the partition axis with contiguous DMAs.
  * Q/K kept transposed in SBUF (qts/kts: [128(d of 2 heads), 512] tiles);
    V in normal layout ([128(s), 64+1] tiles, ones column appended so the
    PV matmul also accumulates the softmax denominator in PSUM row 64).
  * Logits computed transposed, lg[kv, q] = K @ Q.T, both heads packed
    into PE row-groups (tile_position rows 0/64) writing separate banks.
  * exp on ScalarE over [128, 1024] PSUM->SBUF (scale folded in); no
    max-subtraction (logits ~ N(0,1)). Causal masking multiplies the 4
    diagonal-band blocks per q-tile by 0/1 masks.
  * Normalize via reciprocal_approx_fast + stream_shuffle broadcast.
  * The per-q-tile QKV projections and o_proj matmuls are interleaved as
    PE "filler" work between attention pairs so the PE never idles long
    enough for the HAM clock gate to re-throttle it.
"""

import math
import sys
from collections import deque

import numpy as np

for _p in ("/opt/trn_rl_repo", "/opt/trn_rl_repo/concourse"):
    if _p not in sys.path:
        sys.path.insert(0, _p)

import ml_dtypes

BF16 = ml_dtypes.bfloat16

S = 4096
E = 1024
H = 16
D = 64
NCORES = 8
DH = 128  # head dims per core (2 heads x 64)
QT = 512  # query tile (free dim of logits matmuls)
NQ = S // QT  # 8
KB = 128  # kv block (partition dim of logits tiles)
SCALE = 1.0 / math.sqrt(D)

_CACHE = {}


def _build_nc():
    import concourse.tile as tile
    from concourse import bacc, mybir

    dt = mybir.dt
    f32 = dt.float32
    bf16 = dt.bfloat16
    Exp = mybir.ActivationFunctionType.Exp

    nc = bacc.Bacc("TRN2", target_bir_lowering=False, debug=False, num_devices=NCORES)

    xT_d = nc.dram_tensor("xT", [E, S], bf16, kind="ExternalInput")
    wq_d = nc.dram_tensor("wq", [128, 1024], bf16, kind="ExternalInput")
    wk_d = nc.dram_tensor("wk", [128, 1024], bf16, kind="ExternalInput")
    wv_d = nc.dram_tensor("wv", [128, 1024], bf16, kind="ExternalInput")
    wo_d = nc.dram_tensor("wo", [128, 1024], bf16, kind="ExternalInput")
    mask_d = nc.dram_tensor("masks", [KB, 4 * QT], bf16, kind="ExternalInput")
    out_d = nc.dram_tensor("out", [S, E], bf16, kind="ExternalOutput")

    with tile.TileContext(nc) as tc:
        from contextlib import ExitStack

        with ExitStack() as ctx:
            sb = ctx.enter_context(tc.tile_pool(name="sb", bufs=1))
            ps = ctx.enter_context(tc.tile_pool(name="ps", bufs=3, space="PSUM"))
            pvp = ctx.enter_context(tc.tile_pool(name="pvp", bufs=2, space="PSUM"))
            expp = ctx.enter_context(tc.tile_pool(name="expp", bufs=4))
            normp = ctx.enter_context(tc.tile_pool(name="normp", bufs=2))
            ostp = ctx.enter_context(tc.tile_pool(name="ostp", bufs=3))

            # ---- persistent SBUF tensors + input DMA ----
            xts = []
            for ec in range(8):
                t = sb.tile([128, S], bf16, name=f"xt{ec}", tag=f"xt{ec}")
                nc.sync.dma_start(t[:], xT_d[ec * 128 : (ec + 1) * 128, :])
                xts.append(t)

            wq_sb = sb.tile([128, 1024], bf16, name="wq_sb", tag="wq_sb")
            wk_sb = sb.tile([128, 1024], bf16, name="wk_sb", tag="wk_sb")
            wv_sb = sb.tile([128, 1024], bf16, name="wv_sb", tag="wv_sb")
            wo_sb = sb.tile([128, 1024], bf16, name="wo_sb", tag="wo_sb")
            mask_sb = sb.tile([KB, 4 * QT], bf16, name="mask_sb", tag="mask_sb")
            nc.sync.dma_start(wq_sb[:], wq_d[:])
            nc.sync.dma_start(wk_sb[:], wk_d[:])
            nc.sync.dma_start(wv_sb[:], wv_d[:])
            nc.sync.dma_start(wo_sb[:], wo_d[:])
            nc.sync.dma_start(mask_sb[:], mask_d[:])

            kts = [sb.tile([128, QT], bf16, name=f"kt{i}", tag=f"kt{i}") for i in range(NQ)]
            qts = [sb.tile([128, QT], bf16, name=f"qt{i}", tag=f"qt{i}") for i in range(NQ)]
            aots = [sb.tile([128, QT], bf16, name=f"ao{i}", tag=f"ao{i}") for i in range(NQ)]
            vts = [sb.tile([128, 130], bf16, name=f"v{i}", tag=f"v{i}") for i in range(32)]
            for v in vts:
                nc.vector.memset(v[:, 64:65], 1.0)
                nc.vector.memset(v[:, 129:130], 1.0)

            # seed tile for the denominator-reciprocal broadcast
            bcseed = sb.tile([64, QT], f32, name="bcseed", tag="bcseed")
            nc.vector.memset(bcseed[:], 0.0)

            # ---- filler-unit constructors (projections / o_proj) ----
            def kq_units(dst, w, st):
                cols = slice(st * QT, (st + 1) * QT)
                state = {}

                def mm(ec):
                    def f():
                        if ec == 0:
                            state["t"] = ps.tile([128, QT], f32, name="ps_kq", tag="ps")
                        nc.tensor.matmul(
                            state["t"][:],
                            lhsT=w[:, ec * 128 : (ec + 1) * 128],
                            rhs=xts[ec][:, cols],
                            start=(ec == 0),
                            stop=(ec == 7),
                        )

                    return f

                def cast():
                    nc.vector.tensor_copy(dst[:], state["t"][:])

                return [mm(ec) for ec in range(8)] + [cast]

            def v_units(kb):
                state = {}

                def mm(ec):
                    def f():
                        if ec == 0:
                            state["t"] = ps.tile([128, 128], f32, name="ps_v", tag="ps")
                        nc.tensor.matmul(
                            state["t"][:],
                            lhsT=xts[ec][:, kb * 128 : (kb + 1) * 128],
                            rhs=wv_sb[:, ec * 128 : (ec + 1) * 128],
                            start=(ec == 0),
                            stop=(ec == 7),
                        )

                    return f

                def cast():
                    nc.vector.tensor_copy(vts[kb][:, 0:64], state["t"][:, 0:64])
                    nc.vector.tensor_copy(vts[kb][:, 65:129], state["t"][:, 64:128])

                return [mm(ec) for ec in range(8)] + [cast]

            def oproj_units(qj):
                units = []
                for sbi in range(4):
                    for half in range(2):

                        def f(sbi=sbi, half=half):
                            srow = qj * QT + sbi * 128
                            po = ps.tile([128, 512], f32, name="po", tag="ps")
                            nc.tensor.matmul(
                                po[:],
                                lhsT=aots[qj][:, sbi * 128 : (sbi + 1) * 128],
                                rhs=wo_sb[:, half * 512 : (half + 1) * 512],
                                start=True,
                                stop=True,
                            )
                            ost = ostp.tile([128, 512], bf16, name="ost", tag="ost")
                            nc.vector.tensor_copy(ost[:], po[:])
                            nc.sync.dma_start(
                                out_d[srow : srow + 128, half * 512 : (half + 1) * 512],
                                ost[:],
                            )

                        units.append(f)
                return units

            def proj_units(qi2):
                u = []
                u += kq_units(kts[qi2], wk_sb, qi2)
                u += kq_units(qts[qi2], wq_sb, qi2)
                for kb in range(4 * qi2, 4 * qi2 + 4):
                    u += v_units(kb)
                return u

            # ---- prologue: projections for q-tile 0 (dense PE warmup) ----
            for f in proj_units(0):
                f()

            # ---- main loop over q-tiles ----
            for qi in range(NQ):
                fillers = deque()
                if qi + 1 < NQ:
                    fillers.extend(proj_units(qi + 1))
                if qi >= 1:
                    fillers.extend(oproj_units(qi - 1))

                n_kb = 4 * (qi + 1)
                n_pairs = n_kb // 2
                pvA = pvp.tile([65, QT], f32, name="pvA", tag="pv")
                pvB = pvp.tile([65, QT], f32, name="pvB", tag="pv")
                for kp in range(n_pairs):
                    lgA = ps.tile([128, 2 * QT], f32, name="lgA", tag="ps")
                    lgB = ps.tile([128, 2 * QT], f32, name="lgB", tag="ps")
                    for j in (0, 1):
                        kb = 2 * kp + j
                        kvs = slice((kb % 4) * KB, (kb % 4 + 1) * KB)
                        js = slice(j * QT, (j + 1) * QT)
                        ktile = kts[kb // 4]
                        nc.tensor.matmul(
                            lgA[:, js], lhsT=ktile[0:64, kvs], rhs=qts[qi][0:64, :],
                            start=True, stop=True,
                        )
                        nc.tensor.matmul(
                            lgB[:, js], lhsT=ktile[64:128, kvs], rhs=qts[qi][64:128, :],
                            start=True, stop=True,
                        )
                    expA = expp.tile([128, 2 * QT], bf16, name="expA", tag="exp")
                    expB = expp.tile([128, 2 * QT], bf16, name="expB", tag="exp")
                    nc.scalar.activation(expA[:], lgA[:], Exp, scale=SCALE)
                    nc.scalar.activation(expB[:], lgB[:], Exp, scale=SCALE)

                    # PE filler work while ACT computes exp
                    n_pop = math.ceil(len(fillers) / (n_pairs - kp)) if fillers else 0
                    for _ in range(n_pop):
                        fillers.popleft()()

                    for j in (0, 1):
                        kb = 2 * kp + j
                        js = slice(j * QT, (j + 1) * QT)
                        off = kb - 4 * qi
                        if off >= 0:  # diagonal-band block -> apply 0/1 mask
                            ms = mask_sb[:, off * QT : (off + 1) * QT]
                            nc.vector.tensor_mul(expA[:, js], expA[:, js], ms)
                            nc.vector.tensor_mul(expB[:, js], expB[:, js], ms)
                        nc.tensor.matmul(
                            pvA[:], lhsT=vts[kb][:, 0:65], rhs=expA[:, js],
                            start=(kb == 0), stop=(kb == n_kb - 1),
                            skip_group_check=True,
                        )
                        nc.tensor.matmul(
                            pvB[:], lhsT=vts[kb][:, 65:130], rhs=expB[:, js],
                            start=(kb == 0), stop=(kb == n_kb - 1),
                            skip_group_check=True,
                        )
                while fillers:
                    fillers.popleft()()
                # normalize: aot = pv[0:64] / pv[64]
                for pv, r0 in ((pvA, 0), (pvB, 64)):
                    den_sb = normp.tile([1, QT], f32, name="den_sb", tag="den")
                    nc.vector.tensor_copy(den_sb[:], pv[64:65, :])
                    nc.vector.reciprocal_approx_fast(bcseed[0:1, :], den_sb[:])
                    nc.vector.tensor_copy(bcseed[32:33, :], bcseed[0:1, :])
                    bcast = normp.tile([64, QT], f32, name="bcast", tag="bcast")
                    nc.vector.stream_shuffle(bcast[:], bcseed[:], [0] * 32)
                    nc.vector.tensor_mul(aots[qi][r0 : r0 + 64, :], pv[0:64, :], bcast[:])

            # epilogue: o_proj of the final q-tile
            for f in oproj_units(NQ - 1):
                f()

    nc.compile()
    return nc


def _host_inputs(x, Wq, Wk, Wv, Wo):
    x2 = np.asarray(x, dtype=np.float32).reshape(S, E)
    xT = np.ascontiguousarray(x2.T).astype(BF16)

    # 0/1 masks for the 4 diagonal-band offsets: allowed iff kv + off*128 <= q
    kv = np.arange(KB)[:, None]
    q = np.arange(QT)[None, :]
    masks = np.concatenate(
        [(kv + off * KB <= q).astype(BF16) for off in range(4)], axis=1
    )
    masks = np.ascontiguousarray(masks)

    in_maps = []
    for c in range(NCORES):
        r = slice(128 * c, 128 * (c + 1))

        def pack(wT):  # [1024(e), 128(d)] -> [128(p), ec*128+d]
            return np.ascontiguousarray(
                wT.reshape(8, 128, 128).transpose(1, 0, 2).reshape(128, 1024)
            ).astype(BF16)

        wq_c = pack(np.asarray(Wq, np.float32)[r, :].T)
        wk_c = pack(np.asarray(Wk, np.float32)[r, :].T)
        wv_c = pack(np.asarray(Wv, np.float32)[r, :].T)
        wo_c = np.ascontiguousarray(np.asarray(Wo, np.float32)[:, r].T).astype(BF16)
        in_maps.append(
            {
                "xT": xT,
                "wq": wq_c,
                "wk": wk_c,
                "wv": wv_c,
                "wo": wo_c,
                "masks": masks,
            }
        )
    return in_maps


def _get_nc():
    if "nc" not in _CACHE:
        _CACHE["nc"] = _build_nc()
    return _CACHE["nc"]


def run(x, Wq, Wk, Wv, Wo, trace=False, trace_kwargs=None):
    """Build+run the SPMD kernel; returns (full_output [S,E] f32, BassKernelResults)."""
    from concourse.bass_utils import run_bass_kernel_spmd

    nc = _get_nc()
    in_maps = _host_inputs(x, Wq, Wk, Wv, Wo)
    res = run_bass_kernel_spmd(
        nc,
        in_maps,
        list(range(NCORES)),
        trace=trace,
        **(trace_kwargs or {}),
    )
    out = np.zeros((S, E), dtype=np.float32)
    for c in range(NCORES):
        out += res.results[c]["out"].astype(np.float32)
    return out, res


def kernel(x, Wq, Wk, Wv, Wo):
    out, _ = run(x, Wq, Wk, Wv, Wo)
    return out.reshape(1, S, E).astype(np.float32)
